# revision 13
# baseline (speedup 1.0000x reference)
"""Trainium2 Bass kernel for nn_GNN_69707319214464 (3-layer GIN-style GNN).

Strategy (8 NeuronCores, SPMD):
  * Reformulate each GNN layer: the only irregular op is agg_src = A @ h
    (sum of h[src] over in-edges incl. self-loops).  The edge-encoder and
    degree terms are folded into an augmented dense weight:
        z = [agg_src | agg_ea | deg] @ Weff + bias ;  h' = relu_bn(z) @ w2 + b2
    where agg_ea/deg are layer-invariant edge-attr aggregates.
  * Host: append self-loops, sort edges by (dst-tile, src-half), pad each
    (tile, half) edge list to fixed block counts; shard dst-tiles across the
    8 cores.  The src-half split keeps dma_gather indices within int16.
  * Device, per layer: dma_gather of h[src] rows (512 B each, two gathers per
    tile group — low/high half of the node table), one-hot segment-sum
    matmuls into PSUM (lhsT = gathered rows, rhs = dst-one-hot built on DVE
    via is_equal vs iota), dense MLP on the feature-major aggregate,
    PE-transpose back to row-major, AllGather h between layers.
"""

import numpy as np
from functools import lru_cache

import concourse.bass as bass
import concourse.mybir as mybir
import concourse.tile as tile
from concourse import bacc
from concourse.bass_utils import run_bass_kernel_spmd

P = 128
NCORES = 8
H = 128
DE = 16
DE1 = DE + 1
GRP = 2                      # dst-tiles per dma_gather instruction
BN_EPS = 1e-5
F32 = mybir.dt.float32
I16 = mybir.dt.int16

Relu = mybir.ActivationFunctionType.Relu
Identity = mybir.ActivationFunctionType.Identity


# ----------------------------------------------------------------- host prep

def _fold_weights(enc_w, enc_b, w1, b1, g, be, rm, rv, w2, b2, concat):
    """Fold encoder + BN into one [H+DE+1, 2H] weight and [2H] bias."""
    A = g / np.sqrt(rv + BN_EPS)
    Bb = be - rm * A
    if concat:
        w1_top, w1_bot = w1[:H], w1[H:]
    else:
        w1_top = w1_bot = w1
    Weff = np.concatenate([w1_top, enc_w @ w1_bot, (enc_b @ w1_bot)[None, :]], 0)
    Weff = (Weff * A[None, :]).astype(np.float32)
    bias = (b1 * A + Bb).astype(np.float32)
    return Weff, bias, np.asarray(w2, np.float32), np.asarray(b2, np.float32)


def _wrap16(vals):
    """[n] -> [128, n/16] wrapped-16 layout replicated to 128 partitions."""
    w = vals.reshape(-1, 16).T.astype(np.int16)          # [16, n/16]
    return np.tile(w, (8, 1))                            # [128, n/16]


def _prepare(inputs):
    x = np.ascontiguousarray(np.asarray(inputs["x"], np.float32))
    ei = np.asarray(inputs["edge_index"]).astype(np.int64)
    ea = np.asarray(inputs["edge_attr"], np.float32)
    sli = int(np.asarray(inputs["self_loop_index"]))
    slt = float(np.asarray(inputs["self_loop_type"]))
    N = x.shape[0]

    NT = -(-N // P)
    NT = -(-NT // (NCORES * GRP)) * (NCORES * GRP)
    TPC = NT // NCORES
    NPAD = NT * P
    NPC = TPC * P
    SA = NPAD // 2
    assert SA <= 32767 and NPAD - SA <= 32767

    dst = np.concatenate([ei[0], np.arange(N, dtype=np.int64)])
    src = np.concatenate([ei[1], np.arange(N, dtype=np.int64)])
    sl_row = np.zeros((DE,), np.float32)
    sl_row[sli] = slt
    ea_all = np.concatenate([ea, np.broadcast_to(sl_row, (N, DE))], 0)

    tile_of = dst >> 7
    half = (src >= SA).astype(np.int64)
    order = np.argsort(tile_of * 2 + half, kind="stable")
    dsts = dst[order]
    srcs = src[order]
    ea_s = ea_all[order]
    tile_s = tile_of[order]
    half_s = half[order]

    grp_key = tile_s * 2 + half_s
    cnt = np.bincount(grp_key, minlength=2 * NT)
    cntA, cntB = cnt[0::2], cnt[1::2]
    BA = max(1, int(-(-cntA.max() // P)))
    BB = max(1, int(-(-cntB.max() // P)))
    BT = BA + BB                                   # blocks per tile

    starts = np.zeros(2 * NT, np.int64)
    np.cumsum(cnt[:-1], out=starts[1:])
    pos = np.arange(len(dsts)) - starts[grp_key]
    slot_in_tile = np.where(half_s == 0, pos, BA * P + pos)

    idxA = np.zeros((NT, BA * P), np.int16)
    idxB = np.zeros((NT, BB * P), np.int16)
    dst_pad = np.full((NT, BT * P), -1.0, np.float32)
    ea_pad = np.zeros((NT, BT * P, DE1), np.float32)

    selA = half_s == 0
    idxA[tile_s[selA], pos[selA]] = srcs[selA].astype(np.int16)
    selB = ~selA
    idxB[tile_s[selB], pos[selB]] = (srcs[selB] - SA).astype(np.int16)
    dst_pad[tile_s, slot_in_tile] = (dsts & 127).astype(np.float32)
    ea_pad[tile_s, slot_in_tile, :DE] = ea_s
    ea_pad[tile_s, slot_in_tile, DE] = 1.0

    dst_arr = np.ascontiguousarray(
        dst_pad.reshape(NCORES, TPC * BT, P).transpose(0, 2, 1))
    ea_arr = np.ascontiguousarray(
        ea_pad.reshape(NCORES, TPC * BT, P, DE1).transpose(0, 2, 1, 3)
        .reshape(NCORES, P, TPC * BT * DE1))

    # wrapped int16 index arrays, one contiguous chunk per gather group
    NG = TPC // GRP
    idxA_c = idxA.reshape(NCORES, NG, GRP * BA * P)
    idxB_c = idxB.reshape(NCORES, NG, GRP * BB * P)
    idxA_arr = np.zeros((NCORES, P, NG * GRP * BA * 8), np.int16)
    idxB_arr = np.zeros((NCORES, P, NG * GRP * BB * 8), np.int16)
    for c in range(NCORES):
        for g in range(NG):
            idxA_arr[c, :, g * GRP * BA * 8:(g + 1) * GRP * BA * 8] = \
                _wrap16(idxA_c[c, g])
            idxB_arr[c, :, g * GRP * BB * 8:(g + 1) * GRP * BB * 8] = \
                _wrap16(idxB_c[c, g])

    # weights
    w_all, bias_cols = [], []
    Wl, b1l, w2l, b2l = _fold_weights(
        np.asarray(inputs["enc_w0"], np.float32), np.asarray(inputs["enc_b0"], np.float32),
        np.asarray(inputs["w1_0"], np.float32), np.asarray(inputs["b1_0"], np.float32),
        np.asarray(inputs["g0"], np.float32), np.asarray(inputs["be0"], np.float32),
        np.asarray(inputs["rm0"], np.float32), np.asarray(inputs["rv0"], np.float32),
        np.asarray(inputs["w2_0"], np.float32), np.asarray(inputs["b2_0"], np.float32),
        concat=False)
    w_all.append((Wl, w2l))
    bias_cols.append(np.stack([b1l[:H], b1l[H:], b2l], 1))
    for i in range(2):
        Wl, b1l, w2l, b2l = _fold_weights(
            np.asarray(inputs["enc_w"], np.float32)[i], np.asarray(inputs["enc_b"], np.float32)[i],
            np.asarray(inputs["w1"], np.float32)[i], np.asarray(inputs["b1"], np.float32)[i],
            np.asarray(inputs["g"], np.float32)[i], np.asarray(inputs["be"], np.float32)[i],
            np.asarray(inputs["rm"], np.float32)[i], np.asarray(inputs["rv"], np.float32)[i],
            np.asarray(inputs["w2"], np.float32)[i], np.asarray(inputs["b2"], np.float32)[i],
            concat=True)
        w_all.append((Wl, w2l))
        bias_cols.append(np.stack([b1l[:H], b1l[H:], b2l], 1))

    wef = np.stack([w[0] for w in w_all])
    w2f = np.stack([w[1] for w in w_all])
    biasf = np.stack(bias_cols)

    x_pad = np.zeros((NPAD, H), np.float32)
    x_pad[:N] = x

    iota = np.broadcast_to(
        np.tile(np.arange(P, dtype=np.float32), BT), (P, BT * P)).copy()
    ident = np.eye(P, dtype=np.float32)

    in_maps = []
    for c in range(NCORES):
        in_maps.append({
            "x": x_pad,
            "idxA": idxA_arr[c],
            "idxB": idxB_arr[c],
            "dst_loc": dst_arr[c],
            "ea17": ea_arr[c],
            "wef": wef,
            "w2f": w2f,
            "biasf": biasf,
            "iota": iota,
            "ident": ident,
        })
    return dict(in_maps=in_maps, N=N, TPC=TPC, BA=BA, BB=BB, NPAD=NPAD, NPC=NPC)


# ------------------------------------------------------------- bass program

@lru_cache(maxsize=4)
def _build_program(TPC, BA, BB, NPAD, NLAYERS=3, USE_CC=True, PH2=2):
    BT = BA + BB
    NPC = TPC * P
    NG = TPC // GRP
    SA = NPAD // 2
    NW = -(-NPC // 512)

    nc = bacc.Bacc("TRN2", target_bir_lowering=False, debug=False,
                   num_devices=NCORES)

    x_d = nc.dram_tensor("x", [NPAD, H], F32, kind="ExternalInput")
    ia_d = nc.dram_tensor("idxA", [P, NG * GRP * BA * 8], I16, kind="ExternalInput")
    ib_d = nc.dram_tensor("idxB", [P, NG * GRP * BB * 8], I16, kind="ExternalInput")
    dl_d = nc.dram_tensor("dst_loc", [P, TPC * BT], F32, kind="ExternalInput")
    ea_d = nc.dram_tensor("ea17", [P, TPC * BT * DE1], F32, kind="ExternalInput")
    wef_d = nc.dram_tensor("wef", [3, H + DE1, 2 * H], F32, kind="ExternalInput")
    w2_d = nc.dram_tensor("w2f", [3, 2 * H, H], F32, kind="ExternalInput")
    bf_d = nc.dram_tensor("biasf", [3, P, 3], F32, kind="ExternalInput")
    io_d = nc.dram_tensor("iota", [P, BT * P], F32, kind="ExternalInput")
    id_d = nc.dram_tensor("ident", [P, P], F32, kind="ExternalInput")
    out_d = nc.dram_tensor("outT", [P, NPC], F32, kind="ExternalOutput")

    with tile.TileContext(nc) as tc:
        with (
            tc.tile_pool(name="const", bufs=1) as cpool,
            tc.tile_pool(name="wpool", bufs=2) as wpool,
            tc.tile_pool(name="agg", bufs=1) as apool,
            tc.tile_pool(name="gather", bufs=2) as gpool,
            tc.tile_pool(name="eap", bufs=2) as eapool,
            tc.tile_pool(name="onehot", bufs=2) as opool,
            tc.tile_pool(name="dense", bufs=2) as dpool,
            tc.tile_pool(name="psA", bufs=2, space="PSUM") as psa,
            tc.tile_pool(name="psD", bufs=2, space="PSUM") as psd,
            tc.tile_pool(name="dram", bufs=1, space="DRAM") as drpool,
        ):
            idxA_sb = cpool.tile([P, NG * GRP * BA * 8], I16)
            nc.sync.dma_start(idxA_sb[:], ia_d[:])
            idxB_sb = cpool.tile([P, NG * GRP * BB * 8], I16)
            nc.sync.dma_start(idxB_sb[:], ib_d[:])
            dst_loc_sb = cpool.tile([P, TPC * BT], F32)
            nc.sync.dma_start(dst_loc_sb[:], dl_d[:])
            iota_sb = cpool.tile([P, BT * P], F32)
            nc.sync.dma_start(iota_sb[:], io_d[:])
            ident_sb = cpool.tile([P, P], F32)
            nc.sync.dma_start(ident_sb[:], id_d[:])

            aggT = apool.tile([P, NPC], F32)
            aggE = apool.tile([DE1, NPC], F32)

            aspace = "Shared" if USE_CC else "Local"
            h_own0 = drpool.tile([NPC, H], F32)
            h_own1 = drpool.tile([NPC, H], F32)
            h_owns = [h_own0, h_own1]
            h_full0 = drpool.tile([NPAD, H], F32, addr_space=aspace)
            h_full1 = drpool.tile([NPAD, H], F32, addr_space=aspace)
            h_fulls = [h_full0, h_full1]

            for l in range(NLAYERS):
                src_ap = x_d[:] if l == 0 else h_fulls[l - 1][:]
                h_own = h_owns[l] if l < 2 else None
                h_full = h_fulls[l] if l < 2 else None

                wef_hi = wpool.tile([P, 2 * H], F32, tag="wef_hi")
                nc.sync.dma_start(wef_hi[:], wef_d[l, 0:P, :])
                wef_lo = wpool.tile([DE1, 2 * H], F32, tag="wef_lo")
                nc.sync.dma_start(wef_lo[:], wef_d[l, P:P + DE1, :])
                w2a = wpool.tile([P, H], F32, tag="w2a")
                nc.sync.dma_start(w2a[:], w2_d[l, 0:P, :])
                w2b = wpool.tile([P, H], F32, tag="w2b")
                nc.sync.dma_start(w2b[:], w2_d[l, P:2 * P, :])
                bsb = wpool.tile([P, 3], F32, tag="bsb")
                nc.sync.dma_start(bsb[:], bf_d[l, :, :])

                # ---- phase 1: gather + one-hot segment-sum into aggT ----
                for g in range(NG):
                    gbA = gpool.tile([P, GRP * BA * P], F32, tag="gbA")
                    nc.gpsimd.dma_gather(
                        out_ap=gbA[:].rearrange("p (n k) -> p n k", k=P),
                        in_ap=src_ap[0:SA, :],
                        idxs_ap=idxA_sb[:, g * GRP * BA * 8:(g + 1) * GRP * BA * 8],
                        num_idxs=GRP * BA * P,
                        num_idxs_reg=GRP * BA * P,
                        elem_size=H,
                        single_packet=False,
                    )
                    gbB = gpool.tile([P, GRP * BB * P], F32, tag="gbB")
                    nc.gpsimd.dma_gather(
                        out_ap=gbB[:].rearrange("p (n k) -> p n k", k=P),
                        in_ap=src_ap[SA:NPAD, :],
                        idxs_ap=idxB_sb[:, g * GRP * BB * 8:(g + 1) * GRP * BB * 8],
                        num_idxs=GRP * BB * P,
                        num_idxs_reg=GRP * BB * P,
                        elem_size=H,
                        single_packet=False,
                    )
                    for ti in range(GRP):
                        t = g * GRP + ti
                        ob = opool.tile([P, BT * P], F32, tag="ob")
                        nc.vector.tensor_tensor(
                            out=ob[:].rearrange("p (b k) -> p b k", k=P),
                            in0=iota_sb[:].rearrange("p (b k) -> p b k", k=P),
                            in1=dst_loc_sb[:, t * BT:(t + 1) * BT]
                                .to_broadcast([P, BT, P]),
                            op=mybir.AluOpType.is_equal,
                        )
                        ps = psa.tile([P, P], F32, tag="ps")
                        for j in range(BA):
                            nc.tensor.matmul(
                                out=ps[:],
                                lhsT=gbA[:, (ti * BA + j) * P:(ti * BA + j + 1) * P],
                                rhs=ob[:, j * P:(j + 1) * P],
                                start=(j == 0), stop=False)
                        for j in range(BB):
                            nc.tensor.matmul(
                                out=ps[:],
                                lhsT=gbB[:, (ti * BB + j) * P:(ti * BB + j + 1) * P],
                                rhs=ob[:, (BA + j) * P:(BA + j + 1) * P],
                                start=False, stop=(j == BB - 1))
                        nc.scalar.copy(out=aggT[:, t * P:(t + 1) * P], in_=ps[:])
                        if l == 0:
                            eb = eapool.tile([P, BT * DE1], F32, tag="eb")
                            nc.sync.dma_start(
                                eb[:], ea_d[:, t * BT * DE1:(t + 1) * BT * DE1])
                            pse = psa.tile([DE1, P], F32, tag="pse", bufs=1)
                            for j in range(BT):
                                nc.tensor.matmul(
                                    out=pse[:],
                                    lhsT=eb[:, j * DE1:(j + 1) * DE1],
                                    rhs=ob[:, j * P:(j + 1) * P],
                                    start=(j == 0), stop=(j == BT - 1))
                            nc.scalar.copy(out=aggE[:, t * P:(t + 1) * P], in_=pse[:])

                # ---- phase 2: dense MLP (feature-major) ----
                if PH2 == 0:
                    nc.sync.dma_start(out_d[:], aggT[:])
                    continue
                for w in range(NW):
                    c0 = w * 512
                    cw = min(512, NPC - c0)
                    ys = []
                    for half in range(2):
                        psz = psd.tile([P, 512], F32, tag="psz")
                        nc.tensor.matmul(
                            out=psz[:, :cw],
                            lhsT=wef_hi[:, half * P:(half + 1) * P],
                            rhs=aggT[:, c0:c0 + cw],
                            start=True, stop=(PH2 == 1))
                        if PH2 >= 2:
                            nc.tensor.matmul(
                                out=psz[:, :cw],
                                lhsT=wef_lo[:, half * P:(half + 1) * P],
                                rhs=aggE[:, c0:c0 + cw],
                                start=False, stop=True)
                        y = dpool.tile([P, 512], F32, tag=f"y{half}")
                        nc.scalar.activation(
                            out=y[:, :cw], in_=psz[:, :cw], func=Relu,
                            bias=bsb[:, half:half + 1], scale=1.0)
                        ys.append(y)
                    psh = psd.tile([P, 512], F32, tag="psh", bufs=1)
                    nc.tensor.matmul(out=psh[:, :cw], lhsT=w2a[:],
                                     rhs=ys[0][:, :cw], start=True, stop=False)
                    nc.tensor.matmul(out=psh[:, :cw], lhsT=w2b[:],
                                     rhs=ys[1][:, :cw], start=False, stop=True)
                    hT = dpool.tile([P, 512], F32, tag="hT")
                    nc.scalar.activation(
                        out=hT[:, :cw], in_=psh[:, :cw],
                        func=(Relu if l < 2 else Identity),
                        bias=bsb[:, 2:3], scale=1.0)
                    if l == NLAYERS - 1:
                        nc.sync.dma_start(out_d[:, c0:c0 + cw], hT[:, :cw])
                    else:
                        for s in range(cw // P):
                            pst = psd.tile([P, P], F32, tag="pst", bufs=1)
                            nc.tensor.transpose(
                                out=pst[:], in_=hT[:, s * P:(s + 1) * P],
                                identity=ident_sb[:])
                            hr = dpool.tile([P, P], F32, tag="hr")
                            nc.scalar.copy(out=hr[:], in_=pst[:])
                            nc.sync.dma_start(
                                h_own[c0 + s * P:c0 + (s + 1) * P, :], hr[:])
                if l < NLAYERS - 1 and not USE_CC:
                    # debug mode: no collective; results wrong but runnable
                    nc.sync.dma_start(h_full[0:NPC, :], h_own[:])
                if l < NLAYERS - 1 and USE_CC:
                    nc.gpsimd.collective_compute(
                        "AllGather",
                        mybir.AluOpType.bypass,
                        replica_groups=[list(range(NCORES))],
                        ins=[h_own.opt()],
                        outs=[h_full.opt()],
                    )

    nc.compile()
    return nc


# ------------------------------------------------------------------- driver

_LAST_EXEC_NS = None


def kernel(**inputs) -> np.ndarray:
    global _LAST_EXEC_NS
    prep = _prepare(inputs)
    nc = _build_program(prep["TPC"], prep["BA"], prep["BB"], prep["NPAD"])
    res = run_bass_kernel_spmd(nc, prep["in_maps"], list(range(NCORES)))
    _LAST_EXEC_NS = res.exec_time_ns
    out = np.concatenate(
        [np.asarray(res.results[c]["outT"]).T for c in range(NCORES)], 0)
    return out[:prep["N"]].astype(np.float32)


# revision 14
# speedup vs baseline: 1.2336x; 1.2336x over previous
"""Trainium2 Bass kernel for nn_GNN_69707319214464 (3-layer GIN-style GNN).

Strategy (8 NeuronCores, SPMD):
  * Reformulate each GNN layer: the only irregular op is agg_src = A @ h
    (sum of h[src] over in-edges incl. self-loops).  The edge-encoder and
    degree terms are folded into an augmented dense weight:
        z = [agg_src | agg_ea | deg] @ Weff + bias ;  h' = relu_bn(z) @ w2 + b2
    where agg_ea/deg are layer-invariant edge-attr aggregates.
  * Host: append self-loops, sort edges by (dst-tile, src-half), pad each
    (tile, half) edge list to fixed block counts; shard dst-tiles across the
    8 cores.  The src-half split keeps dma_gather indices within int16.
  * Device, per layer: dma_gather of h[src] rows (512 B each, two gathers per
    tile group — low/high half of the node table), one-hot segment-sum
    matmuls into PSUM (lhsT = gathered rows, rhs = dst-one-hot built on DVE
    via is_equal vs iota), dense MLP on the feature-major aggregate,
    PE-transpose back to row-major, AllGather h between layers.
"""

import numpy as np
from functools import lru_cache

import concourse.bass as bass
import concourse.mybir as mybir
import concourse.tile as tile
from concourse import bacc
from concourse.bass_utils import run_bass_kernel_spmd

P = 128
NCORES = 8
H = 128
DE = 16
DE1 = DE + 1
GRP = 2                      # dst-tiles per dma_gather instruction
BN_EPS = 1e-5
F32 = mybir.dt.float32
I16 = mybir.dt.int16

Relu = mybir.ActivationFunctionType.Relu
Identity = mybir.ActivationFunctionType.Identity


# ----------------------------------------------------------------- host prep

def _fold_weights(enc_w, enc_b, w1, b1, g, be, rm, rv, w2, b2, concat):
    """Fold encoder + BN into one [H+DE+1, 2H] weight and [2H] bias."""
    A = g / np.sqrt(rv + BN_EPS)
    Bb = be - rm * A
    if concat:
        w1_top, w1_bot = w1[:H], w1[H:]
    else:
        w1_top = w1_bot = w1
    Weff = np.concatenate([w1_top, enc_w @ w1_bot, (enc_b @ w1_bot)[None, :]], 0)
    Weff = (Weff * A[None, :]).astype(np.float32)
    bias = (b1 * A + Bb).astype(np.float32)
    return Weff, bias, np.asarray(w2, np.float32), np.asarray(b2, np.float32)


def _wrap16(vals):
    """[n] -> [128, n/16] wrapped-16 layout replicated to 128 partitions."""
    w = vals.reshape(-1, 16).T.astype(np.int16)          # [16, n/16]
    return np.tile(w, (8, 1))                            # [128, n/16]


def _prepare(inputs):
    x = np.ascontiguousarray(np.asarray(inputs["x"], np.float32))
    ei = np.asarray(inputs["edge_index"]).astype(np.int64)
    ea = np.asarray(inputs["edge_attr"], np.float32)
    sli = int(np.asarray(inputs["self_loop_index"]))
    slt = float(np.asarray(inputs["self_loop_type"]))
    N = x.shape[0]

    NT = -(-N // P)
    NT = -(-NT // (NCORES * GRP)) * (NCORES * GRP)
    TPC = NT // NCORES
    NPAD = NT * P
    NPC = TPC * P
    SA = NPAD // 2
    assert SA <= 32767 and NPAD - SA <= 32767

    dst = np.concatenate([ei[0], np.arange(N, dtype=np.int64)])
    src = np.concatenate([ei[1], np.arange(N, dtype=np.int64)])
    sl_row = np.zeros((DE,), np.float32)
    sl_row[sli] = slt
    ea_all = np.concatenate([ea, np.broadcast_to(sl_row, (N, DE))], 0)

    tile_of = dst >> 7
    half = (src >= SA).astype(np.int64)
    order = np.argsort(tile_of * 2 + half, kind="stable")
    dsts = dst[order]
    srcs = src[order]
    ea_s = ea_all[order]
    tile_s = tile_of[order]
    half_s = half[order]

    grp_key = tile_s * 2 + half_s
    cnt = np.bincount(grp_key, minlength=2 * NT)
    cntA, cntB = cnt[0::2], cnt[1::2]
    BA = max(1, int(-(-cntA.max() // P)))
    BB = max(1, int(-(-cntB.max() // P)))
    BT = BA + BB                                   # blocks per tile

    starts = np.zeros(2 * NT, np.int64)
    np.cumsum(cnt[:-1], out=starts[1:])
    pos = np.arange(len(dsts)) - starts[grp_key]
    slot_in_tile = np.where(half_s == 0, pos, BA * P + pos)

    idxA = np.zeros((NT, BA * P), np.int16)
    idxB = np.zeros((NT, BB * P), np.int16)
    dst_pad = np.full((NT, BT * P), -1.0, np.float32)
    ea_pad = np.zeros((NT, BT * P, DE1), np.float32)

    selA = half_s == 0
    idxA[tile_s[selA], pos[selA]] = srcs[selA].astype(np.int16)
    selB = ~selA
    idxB[tile_s[selB], pos[selB]] = (srcs[selB] - SA).astype(np.int16)
    dst_pad[tile_s, slot_in_tile] = (dsts & 127).astype(np.float32)
    ea_pad[tile_s, slot_in_tile, :DE] = ea_s
    ea_pad[tile_s, slot_in_tile, DE] = 1.0

    dst_arr = np.ascontiguousarray(
        dst_pad.reshape(NCORES, TPC * BT, P).transpose(0, 2, 1))
    ea_arr = np.ascontiguousarray(
        ea_pad.reshape(NCORES, TPC * BT, P, DE1).transpose(0, 2, 1, 3)
        .reshape(NCORES, P, TPC * BT * DE1))

    # wrapped int16 index arrays, one contiguous chunk per gather group
    NG = TPC // GRP
    idxA_c = idxA.reshape(NCORES, NG, GRP * BA * P)
    idxB_c = idxB.reshape(NCORES, NG, GRP * BB * P)
    idxA_arr = np.zeros((NCORES, P, NG * GRP * BA * 8), np.int16)
    idxB_arr = np.zeros((NCORES, P, NG * GRP * BB * 8), np.int16)
    for c in range(NCORES):
        for g in range(NG):
            idxA_arr[c, :, g * GRP * BA * 8:(g + 1) * GRP * BA * 8] = \
                _wrap16(idxA_c[c, g])
            idxB_arr[c, :, g * GRP * BB * 8:(g + 1) * GRP * BB * 8] = \
                _wrap16(idxB_c[c, g])

    # weights
    w_all, bias_cols = [], []
    Wl, b1l, w2l, b2l = _fold_weights(
        np.asarray(inputs["enc_w0"], np.float32), np.asarray(inputs["enc_b0"], np.float32),
        np.asarray(inputs["w1_0"], np.float32), np.asarray(inputs["b1_0"], np.float32),
        np.asarray(inputs["g0"], np.float32), np.asarray(inputs["be0"], np.float32),
        np.asarray(inputs["rm0"], np.float32), np.asarray(inputs["rv0"], np.float32),
        np.asarray(inputs["w2_0"], np.float32), np.asarray(inputs["b2_0"], np.float32),
        concat=False)
    w_all.append((Wl, w2l))
    bias_cols.append(np.stack([b1l[:H], b1l[H:], b2l], 1))
    for i in range(2):
        Wl, b1l, w2l, b2l = _fold_weights(
            np.asarray(inputs["enc_w"], np.float32)[i], np.asarray(inputs["enc_b"], np.float32)[i],
            np.asarray(inputs["w1"], np.float32)[i], np.asarray(inputs["b1"], np.float32)[i],
            np.asarray(inputs["g"], np.float32)[i], np.asarray(inputs["be"], np.float32)[i],
            np.asarray(inputs["rm"], np.float32)[i], np.asarray(inputs["rv"], np.float32)[i],
            np.asarray(inputs["w2"], np.float32)[i], np.asarray(inputs["b2"], np.float32)[i],
            concat=True)
        w_all.append((Wl, w2l))
        bias_cols.append(np.stack([b1l[:H], b1l[H:], b2l], 1))

    wef = np.stack([w[0] for w in w_all])
    w2f = np.stack([w[1] for w in w_all])
    biasf = np.stack(bias_cols)

    x_pad = np.zeros((NPAD, H), np.float32)
    x_pad[:N] = x

    iota = np.broadcast_to(
        np.tile(np.arange(P, dtype=np.float32), BT), (P, BT * P)).copy()
    ident = np.eye(P, dtype=np.float32)

    in_maps = []
    for c in range(NCORES):
        in_maps.append({
            "x": x_pad,
            "idxA": idxA_arr[c],
            "idxB": idxB_arr[c],
            "dst_loc": dst_arr[c],
            "ea17": ea_arr[c],
            "wef": wef,
            "w2f": w2f,
            "biasf": biasf,
            "iota": iota,
            "ident": ident,
        })
    return dict(in_maps=in_maps, N=N, TPC=TPC, BA=BA, BB=BB, NPAD=NPAD, NPC=NPC)


# ------------------------------------------------------------- bass program

@lru_cache(maxsize=4)
def _build_program(TPC, BA, BB, NPAD, NLAYERS=3, USE_CC=True, PH2=2):
    BT = BA + BB
    NPC = TPC * P
    NG = TPC // GRP
    SA = NPAD // 2
    NW = -(-NPC // 512)

    nc = bacc.Bacc("TRN2", target_bir_lowering=False, debug=False,
                   num_devices=NCORES, num_swdge_queues=4)

    x_d = nc.dram_tensor("x", [NPAD, H], F32, kind="ExternalInput")
    ia_d = nc.dram_tensor("idxA", [P, NG * GRP * BA * 8], I16, kind="ExternalInput")
    ib_d = nc.dram_tensor("idxB", [P, NG * GRP * BB * 8], I16, kind="ExternalInput")
    dl_d = nc.dram_tensor("dst_loc", [P, TPC * BT], F32, kind="ExternalInput")
    ea_d = nc.dram_tensor("ea17", [P, TPC * BT * DE1], F32, kind="ExternalInput")
    wef_d = nc.dram_tensor("wef", [3, H + DE1, 2 * H], F32, kind="ExternalInput")
    w2_d = nc.dram_tensor("w2f", [3, 2 * H, H], F32, kind="ExternalInput")
    bf_d = nc.dram_tensor("biasf", [3, P, 3], F32, kind="ExternalInput")
    io_d = nc.dram_tensor("iota", [P, BT * P], F32, kind="ExternalInput")
    id_d = nc.dram_tensor("ident", [P, P], F32, kind="ExternalInput")
    out_d = nc.dram_tensor("outT", [P, NPC], F32, kind="ExternalOutput")

    with tile.TileContext(nc) as tc:
        with (
            tc.tile_pool(name="const", bufs=1) as cpool,
            tc.tile_pool(name="wpool", bufs=2) as wpool,
            tc.tile_pool(name="agg", bufs=1) as apool,
            tc.tile_pool(name="gather", bufs=2) as gpool,
            tc.tile_pool(name="eap", bufs=2) as eapool,
            tc.tile_pool(name="onehot", bufs=2) as opool,
            tc.tile_pool(name="dense", bufs=2) as dpool,
            tc.tile_pool(name="psA", bufs=2, space="PSUM") as psa,
            tc.tile_pool(name="psD", bufs=2, space="PSUM") as psd,
            tc.tile_pool(name="dram", bufs=1, space="DRAM") as drpool,
        ):
            idxA_sb = cpool.tile([P, NG * GRP * BA * 8], I16)
            nc.sync.dma_start(idxA_sb[:], ia_d[:])
            idxB_sb = cpool.tile([P, NG * GRP * BB * 8], I16)
            nc.sync.dma_start(idxB_sb[:], ib_d[:])
            dst_loc_sb = cpool.tile([P, TPC * BT], F32)
            nc.sync.dma_start(dst_loc_sb[:], dl_d[:])
            iota_sb = cpool.tile([P, BT * P], F32)
            nc.sync.dma_start(iota_sb[:], io_d[:])
            ident_sb = cpool.tile([P, P], F32)
            nc.sync.dma_start(ident_sb[:], id_d[:])

            aggT = apool.tile([P, NPC], F32)
            aggE = apool.tile([DE1, NPC], F32)

            aspace = "Shared" if USE_CC else "Local"
            h_own0 = drpool.tile([NPC, H], F32)
            h_own1 = drpool.tile([NPC, H], F32)
            h_owns = [h_own0, h_own1]
            h_full0 = drpool.tile([NPAD, H], F32, addr_space=aspace)
            h_full1 = drpool.tile([NPAD, H], F32, addr_space=aspace)
            h_fulls = [h_full0, h_full1]

            for l in range(NLAYERS):
                src_ap = x_d[:] if l == 0 else h_fulls[l - 1][:]
                h_own = h_owns[l] if l < 2 else None
                h_full = h_fulls[l] if l < 2 else None

                wef_hi = wpool.tile([P, 2 * H], F32, tag="wef_hi")
                nc.sync.dma_start(wef_hi[:], wef_d[l, 0:P, :])
                wef_lo = wpool.tile([DE1, 2 * H], F32, tag="wef_lo")
                nc.sync.dma_start(wef_lo[:], wef_d[l, P:P + DE1, :])
                w2a = wpool.tile([P, H], F32, tag="w2a")
                nc.sync.dma_start(w2a[:], w2_d[l, 0:P, :])
                w2b = wpool.tile([P, H], F32, tag="w2b")
                nc.sync.dma_start(w2b[:], w2_d[l, P:2 * P, :])
                bsb = wpool.tile([P, 3], F32, tag="bsb")
                nc.sync.dma_start(bsb[:], bf_d[l, :, :])

                # ---- phase 1: gather + one-hot segment-sum into aggT ----
                for g in range(NG):
                    qa = (2 * g) % 4
                    qb = (2 * g + 1) % 4
                    gbA = gpool.tile([P, GRP * BA * P], F32, tag="gbA")
                    nc.gpsimd.dma_gather(
                        out_ap=gbA[:].rearrange("p (n k) -> p n k", k=P),
                        in_ap=src_ap[0:SA, :],
                        idxs_ap=idxA_sb[:, g * GRP * BA * 8:(g + 1) * GRP * BA * 8],
                        num_idxs=GRP * BA * P,
                        num_idxs_reg=GRP * BA * P,
                        elem_size=H,
                        single_packet=False,
                        queue_num=qa,
                    )
                    gbB = gpool.tile([P, GRP * BB * P], F32, tag="gbB")
                    nc.gpsimd.dma_gather(
                        out_ap=gbB[:].rearrange("p (n k) -> p n k", k=P),
                        in_ap=src_ap[SA:NPAD, :],
                        idxs_ap=idxB_sb[:, g * GRP * BB * 8:(g + 1) * GRP * BB * 8],
                        num_idxs=GRP * BB * P,
                        num_idxs_reg=GRP * BB * P,
                        elem_size=H,
                        single_packet=False,
                        queue_num=qb,
                    )
                    for ti in range(GRP):
                        t = g * GRP + ti
                        ob = opool.tile([P, BT * P], F32, tag="ob")
                        nc.vector.tensor_tensor(
                            out=ob[:].rearrange("p (b k) -> p b k", k=P),
                            in0=iota_sb[:].rearrange("p (b k) -> p b k", k=P),
                            in1=dst_loc_sb[:, t * BT:(t + 1) * BT]
                                .to_broadcast([P, BT, P]),
                            op=mybir.AluOpType.is_equal,
                        )
                        ps = psa.tile([P, P], F32, tag="ps")
                        for j in range(BA):
                            nc.tensor.matmul(
                                out=ps[:],
                                lhsT=gbA[:, (ti * BA + j) * P:(ti * BA + j + 1) * P],
                                rhs=ob[:, j * P:(j + 1) * P],
                                start=(j == 0), stop=False)
                        for j in range(BB):
                            nc.tensor.matmul(
                                out=ps[:],
                                lhsT=gbB[:, (ti * BB + j) * P:(ti * BB + j + 1) * P],
                                rhs=ob[:, (BA + j) * P:(BA + j + 1) * P],
                                start=False, stop=(j == BB - 1))
                        nc.scalar.copy(out=aggT[:, t * P:(t + 1) * P], in_=ps[:])
                        if l == 0:
                            eb = eapool.tile([P, BT * DE1], F32, tag="eb")
                            nc.sync.dma_start(
                                eb[:], ea_d[:, t * BT * DE1:(t + 1) * BT * DE1])
                            pse = psa.tile([DE1, P], F32, tag="pse", bufs=1)
                            for j in range(BT):
                                nc.tensor.matmul(
                                    out=pse[:],
                                    lhsT=eb[:, j * DE1:(j + 1) * DE1],
                                    rhs=ob[:, j * P:(j + 1) * P],
                                    start=(j == 0), stop=(j == BT - 1))
                            nc.scalar.copy(out=aggE[:, t * P:(t + 1) * P], in_=pse[:])

                # ---- phase 2: dense MLP (feature-major) ----
                if PH2 == 0:
                    nc.sync.dma_start(out_d[:], aggT[:])
                    continue
                for w in range(NW):
                    c0 = w * 512
                    cw = min(512, NPC - c0)
                    ys = []
                    for half in range(2):
                        psz = psd.tile([P, 512], F32, tag="psz")
                        nc.tensor.matmul(
                            out=psz[:, :cw],
                            lhsT=wef_hi[:, half * P:(half + 1) * P],
                            rhs=aggT[:, c0:c0 + cw],
                            start=True, stop=(PH2 == 1))
                        if PH2 >= 2:
                            nc.tensor.matmul(
                                out=psz[:, :cw],
                                lhsT=wef_lo[:, half * P:(half + 1) * P],
                                rhs=aggE[:, c0:c0 + cw],
                                start=False, stop=True)
                        y = dpool.tile([P, 512], F32, tag=f"y{half}")
                        nc.scalar.activation(
                            out=y[:, :cw], in_=psz[:, :cw], func=Relu,
                            bias=bsb[:, half:half + 1], scale=1.0)
                        ys.append(y)
                    psh = psd.tile([P, 512], F32, tag="psh", bufs=1)
                    nc.tensor.matmul(out=psh[:, :cw], lhsT=w2a[:],
                                     rhs=ys[0][:, :cw], start=True, stop=False)
                    nc.tensor.matmul(out=psh[:, :cw], lhsT=w2b[:],
                                     rhs=ys[1][:, :cw], start=False, stop=True)
                    hT = dpool.tile([P, 512], F32, tag="hT")
                    nc.scalar.activation(
                        out=hT[:, :cw], in_=psh[:, :cw],
                        func=(Relu if l < 2 else Identity),
                        bias=bsb[:, 2:3], scale=1.0)
                    if l == NLAYERS - 1:
                        nc.sync.dma_start(out_d[:, c0:c0 + cw], hT[:, :cw])
                    else:
                        for s in range(cw // P):
                            pst = psd.tile([P, P], F32, tag="pst", bufs=1)
                            nc.tensor.transpose(
                                out=pst[:], in_=hT[:, s * P:(s + 1) * P],
                                identity=ident_sb[:])
                            hr = dpool.tile([P, P], F32, tag="hr")
                            nc.scalar.copy(out=hr[:], in_=pst[:])
                            nc.sync.dma_start(
                                h_own[c0 + s * P:c0 + (s + 1) * P, :], hr[:])
                if l < NLAYERS - 1 and not USE_CC:
                    # debug mode: no collective; results wrong but runnable
                    nc.sync.dma_start(h_full[0:NPC, :], h_own[:])
                if l < NLAYERS - 1 and USE_CC:
                    nc.gpsimd.collective_compute(
                        "AllGather",
                        mybir.AluOpType.bypass,
                        replica_groups=[list(range(NCORES))],
                        ins=[h_own.opt()],
                        outs=[h_full.opt()],
                    )

    nc.compile()
    return nc


# ------------------------------------------------------------------- driver

_LAST_EXEC_NS = None


def kernel(**inputs) -> np.ndarray:
    global _LAST_EXEC_NS
    prep = _prepare(inputs)
    nc = _build_program(prep["TPC"], prep["BA"], prep["BB"], prep["NPAD"])
    res = run_bass_kernel_spmd(nc, prep["in_maps"], list(range(NCORES)))
    _LAST_EXEC_NS = res.exec_time_ns
    out = np.concatenate(
        [np.asarray(res.results[c]["outT"]).T for c in range(NCORES)], 0)
    return out[:prep["N"]].astype(np.float32)


# revision 19
# speedup vs baseline: 2.8449x; 2.3061x over previous
"""Trainium2 Bass kernel for nn_GNN_69707319214464 (3-layer GIN-style GNN).

Strategy (8 NeuronCores, SPMD):
  * Reformulate each GNN layer: the only irregular op is agg_src = A @ h
    (sum of h[src] over in-edges).  Self-loops are peeled off (hT of own
    nodes is kept in SBUF and added directly); the edge-encoder, degree and
    self-loop-attr terms are folded into an augmented dense weight/bias:
        z = [agg_src | agg_ea | deg] @ Weff + bias ;  h' = relu_bn(z) @ w2 + b2
    where agg_ea/deg are layer-invariant edge-attr aggregates.
  * Host: sort edges by (dst-tile, src-half), pad each (tile, half) edge list
    to fixed block counts with trailing -1s (skipped at runtime via
    num_idxs_reg); shard dst-tiles across the 8 cores.  The src-half split
    keeps dma_gather indices within int16.
  * Device, per layer: dma_gather of h[src] rows (bf16, 256 B each, two
    gathers per tile over 4 SWDGE queues), one-hot segment-sum matmuls into
    fp32 PSUM (lhsT = gathered rows, rhs = dst-one-hot built on DVE via
    is_equal vs iota), dense MLP in fp32 on the feature-major aggregate
    (interleaved with phase 1), PE-transpose back to row-major, chunked
    AllGather of h overlapping compute.
"""

import numpy as np
import ml_dtypes
from functools import lru_cache

import concourse.bass as bass
import concourse.mybir as mybir
import concourse.tile as tile
from concourse import bacc
from concourse.bass_utils import run_bass_kernel_spmd

P = 128
NCORES = 8
H = 128
DE = 16
DE1 = DE + 1
BN_EPS = 1e-5
F32 = mybir.dt.float32
BF16 = mybir.dt.bfloat16
I16 = mybir.dt.int16
I32 = mybir.dt.int32
NPBF = ml_dtypes.bfloat16

Relu = mybir.ActivationFunctionType.Relu
Identity = mybir.ActivationFunctionType.Identity


# ----------------------------------------------------------------- host prep

def _fold_weights(enc_w, enc_b, w1, b1, g, be, rm, rv, w2, b2, concat, sl_row17):
    """Fold encoder + BN (+ self-loop attr constant) into [H+DE+1, 2H] + bias."""
    A = g / np.sqrt(rv + BN_EPS)
    Bb = be - rm * A
    if concat:
        w1_top, w1_bot = w1[:H], w1[H:]
    else:
        w1_top = w1_bot = w1
    Weff = np.concatenate([w1_top, enc_w @ w1_bot, (enc_b @ w1_bot)[None, :]], 0)
    Weff = (Weff * A[None, :]).astype(np.float32)
    bias = (b1 * A + Bb).astype(np.float32)
    # self-loop edges are peeled off the edge list; their edge-attr/degree
    # contribution (one [sl_row | 1] per node) folds into the bias
    bias = bias + sl_row17 @ Weff[H:H + DE1]
    return Weff, bias.astype(np.float32), np.asarray(w2, np.float32), \
        np.asarray(b2, np.float32)


def _wrap16(vals):
    """[n] -> [128, n/16] wrapped-16 layout replicated to 128 partitions."""
    w = vals.reshape(-1, 16).T.astype(np.int16)
    return np.tile(w, (8, 1))


def _prepare(inputs):
    x = np.ascontiguousarray(np.asarray(inputs["x"], np.float32))
    ei = np.asarray(inputs["edge_index"]).astype(np.int64)
    ea = np.asarray(inputs["edge_attr"], np.float32)
    sli = int(np.asarray(inputs["self_loop_index"]))
    slt = float(np.asarray(inputs["self_loop_type"]))
    N = x.shape[0]

    NT = -(-N // P)
    NT = -(-NT // NCORES) * NCORES
    TPC = NT // NCORES
    NPAD = NT * P
    NPC = TPC * P
    SA = NPAD // 2
    assert SA <= 32767 and NPAD - SA <= 32767

    # no self-loops in the edge list (peeled off, handled via hT + bias fold)
    dst = ei[0]
    src = ei[1]
    sl_row = np.zeros((DE,), np.float32)
    sl_row[sli] = slt
    sl_row17 = np.concatenate([sl_row, [1.0]]).astype(np.float32)

    tile_of = dst >> 7
    half = (src >= SA).astype(np.int64)
    order = np.argsort(tile_of * 2 + half, kind="stable")
    dsts = dst[order]
    srcs = src[order]
    ea_s = ea[order]
    tile_s = tile_of[order]
    half_s = half[order]

    grp_key = tile_s * 2 + half_s
    cnt = np.bincount(grp_key, minlength=2 * NT)
    cntA, cntB = cnt[0::2].copy(), cnt[1::2].copy()
    BA = max(1, int(-(-cntA.max() // P)))
    BB = max(1, int(-(-cntB.max() // P)))
    BT = BA + BB

    starts = np.zeros(2 * NT, np.int64)
    np.cumsum(cnt[:-1], out=starts[1:])
    pos = np.arange(len(dsts)) - starts[grp_key]
    slot_in_tile = np.where(half_s == 0, pos, BA * P + pos)

    idxA = np.full((NT, BA * P), -1, np.int16)
    idxB = np.full((NT, BB * P), -1, np.int16)
    dst_pad = np.full((NT, BT * P), -1.0, np.float32)
    ea_pad = np.zeros((NT, BT * P, DE1), np.float32)

    selA = half_s == 0
    idxA[tile_s[selA], pos[selA]] = srcs[selA].astype(np.int16)
    selB = ~selA
    idxB[tile_s[selB], pos[selB]] = (srcs[selB] - SA).astype(np.int16)
    dst_pad[tile_s, slot_in_tile] = (dsts & 127).astype(np.float32)
    ea_pad[tile_s, slot_in_tile, :DE] = ea_s
    ea_pad[tile_s, slot_in_tile, DE] = 1.0

    # runtime index counts; ensure >= 16 non-negative per gather
    for idx_, cnt_ in ((idxA, cntA), (idxB, cntB)):
        low = np.where(cnt_ < 16)[0]
        for t in low:
            idx_[t, cnt_[t]:16] = 0
            cnt_[t] = 16

    dst_arr = np.ascontiguousarray(
        dst_pad.reshape(NCORES, TPC * BT, P).transpose(0, 2, 1)).astype(NPBF)
    ea_arr = np.ascontiguousarray(
        ea_pad.reshape(NCORES, TPC * BT, P, DE1).transpose(0, 2, 1, 3)
        .reshape(NCORES, P, TPC * BT * DE1)).astype(NPBF)

    idxA_c = idxA.reshape(NCORES, TPC, BA * P)
    idxB_c = idxB.reshape(NCORES, TPC, BB * P)
    idxA_arr = np.zeros((NCORES, P, TPC * BA * 8), np.int16)
    idxB_arr = np.zeros((NCORES, P, TPC * BB * 8), np.int16)
    for c in range(NCORES):
        for t in range(TPC):
            idxA_arr[c, :, t * BA * 8:(t + 1) * BA * 8] = _wrap16(idxA_c[c, t])
            idxB_arr[c, :, t * BB * 8:(t + 1) * BB * 8] = _wrap16(idxB_c[c, t])

    cnts = np.stack([cntA.reshape(NCORES, TPC),
                     cntB.reshape(NCORES, TPC)], 2).astype(np.int32)  # [8,TPC,2]

    # weights
    w_all, bias_cols = [], []
    Wl, b1l, w2l, b2l = _fold_weights(
        np.asarray(inputs["enc_w0"], np.float32), np.asarray(inputs["enc_b0"], np.float32),
        np.asarray(inputs["w1_0"], np.float32), np.asarray(inputs["b1_0"], np.float32),
        np.asarray(inputs["g0"], np.float32), np.asarray(inputs["be0"], np.float32),
        np.asarray(inputs["rm0"], np.float32), np.asarray(inputs["rv0"], np.float32),
        np.asarray(inputs["w2_0"], np.float32), np.asarray(inputs["b2_0"], np.float32),
        False, sl_row17)
    w_all.append((Wl, w2l))
    bias_cols.append(np.stack([b1l[:H], b1l[H:], b2l], 1))
    for i in range(2):
        Wl, b1l, w2l, b2l = _fold_weights(
            np.asarray(inputs["enc_w"], np.float32)[i], np.asarray(inputs["enc_b"], np.float32)[i],
            np.asarray(inputs["w1"], np.float32)[i], np.asarray(inputs["b1"], np.float32)[i],
            np.asarray(inputs["g"], np.float32)[i], np.asarray(inputs["be"], np.float32)[i],
            np.asarray(inputs["rm"], np.float32)[i], np.asarray(inputs["rv"], np.float32)[i],
            np.asarray(inputs["w2"], np.float32)[i], np.asarray(inputs["b2"], np.float32)[i],
            True, sl_row17)
        w_all.append((Wl, w2l))
        bias_cols.append(np.stack([b1l[:H], b1l[H:], b2l], 1))

    wef = np.stack([w[0] for w in w_all])
    w2f = np.stack([w[1] for w in w_all])
    biasf = np.stack(bias_cols)

    x_pad = np.zeros((NPAD, H), NPBF)
    x_pad[:N] = x.astype(NPBF)
    # transposed own-shard x (bf16) for the peeled self-loop term, per core
    xp32 = np.zeros((NPAD, H), np.float32)
    xp32[:N] = x
    xT = np.zeros((NCORES, P, NPC), NPBF)
    for c in range(NCORES):
        xT[c] = xp32[c * NPC:(c + 1) * NPC].T.astype(NPBF)

    iota = np.broadcast_to(
        np.tile(np.arange(P, dtype=np.float32), BT), (P, BT * P)).astype(NPBF)
    ident = np.eye(P, dtype=np.float32)

    in_maps = []
    for c in range(NCORES):
        in_maps.append({
            "x": x_pad,
            "xT": np.ascontiguousarray(xT[c]),
            "idxA": idxA_arr[c],
            "idxB": idxB_arr[c],
            "cnts": np.ascontiguousarray(cnts[c].reshape(1, TPC * 2)),
            "dst_loc": dst_arr[c],
            "ea17": ea_arr[c],
            "wef": wef,
            "w2f": w2f,
            "biasf": biasf,
            "iota": iota,
            "ident": ident,
        })
    return dict(in_maps=in_maps, N=N, TPC=TPC, BA=BA, BB=BB, NPAD=NPAD, NPC=NPC)


# ------------------------------------------------------------- bass program

@lru_cache(maxsize=4)
def _build_program(TPC, BA, BB, NPAD):
    BT = BA + BB
    NPC = TPC * P
    SA = NPAD // 2
    NW = -(-NPC // 512)

    # AllGather chunks: 4 tile ranges, first three aligned to 4-tile (512-node)
    # dense-window boundaries
    q = TPC // 4 // 4 * 4
    bounds = [0, q, 2 * q, 3 * q, TPC]
    chunk_tiles = [(bounds[i], bounds[i + 1]) for i in range(4)
                   if bounds[i + 1] > bounds[i]]
    chunk_after_w = {}
    for k, (a, b) in enumerate(chunk_tiles):
        w_end = -(-b * P // 512) - 1
        chunk_after_w[w_end] = k

    nc = bacc.Bacc("TRN2", target_bir_lowering=False, debug=False,
                   num_devices=NCORES, num_swdge_queues=4)

    x_d = nc.dram_tensor("x", [NPAD, H], BF16, kind="ExternalInput")
    xT_d = nc.dram_tensor("xT", [P, NPC], BF16, kind="ExternalInput")
    ia_d = nc.dram_tensor("idxA", [P, TPC * BA * 8], I16, kind="ExternalInput")
    ib_d = nc.dram_tensor("idxB", [P, TPC * BB * 8], I16, kind="ExternalInput")
    cn_d = nc.dram_tensor("cnts", [1, TPC * 2], I32, kind="ExternalInput")
    dl_d = nc.dram_tensor("dst_loc", [P, TPC * BT], BF16, kind="ExternalInput")
    ea_d = nc.dram_tensor("ea17", [P, TPC * BT * DE1], BF16, kind="ExternalInput")
    wef_d = nc.dram_tensor("wef", [3, H + DE1, 2 * H], F32, kind="ExternalInput")
    w2_d = nc.dram_tensor("w2f", [3, 2 * H, H], F32, kind="ExternalInput")
    bf_d = nc.dram_tensor("biasf", [3, P, 3], F32, kind="ExternalInput")
    io_d = nc.dram_tensor("iota", [P, BT * P], BF16, kind="ExternalInput")
    id_d = nc.dram_tensor("ident", [P, P], F32, kind="ExternalInput")
    out_d = nc.dram_tensor("outT", [P, NPC], F32, kind="ExternalOutput")

    with tile.TileContext(nc) as tc:
        with (
            tc.tile_pool(name="const", bufs=1) as cpool,
            tc.tile_pool(name="wpool", bufs=2) as wpool,
            tc.tile_pool(name="agg", bufs=1) as apool,
            tc.tile_pool(name="gather", bufs=1) as gpool,
            tc.tile_pool(name="eap", bufs=2) as eapool,
            tc.tile_pool(name="onehot", bufs=3) as opool,
            tc.tile_pool(name="dense", bufs=2) as dpool,
            tc.tile_pool(name="psA", bufs=2, space="PSUM") as psa,
            tc.tile_pool(name="psD", bufs=2, space="PSUM") as psd,
            tc.tile_pool(name="dram", bufs=1, space="DRAM") as drpool,
        ):
            idxA_sb = cpool.tile([P, TPC * BA * 8], I16)
            nc.sync.dma_start(idxA_sb[:], ia_d[:])
            idxB_sb = cpool.tile([P, TPC * BB * 8], I16)
            nc.sync.dma_start(idxB_sb[:], ib_d[:])
            cnt_sb = cpool.tile([1, TPC * 2], I32)
            nc.sync.dma_start(cnt_sb[:], cn_d[:])
            dst_loc_sb = cpool.tile([P, TPC * BT], BF16)
            nc.sync.dma_start(dst_loc_sb[:], dl_d[:])
            iota_sb = cpool.tile([P, BT * P], BF16)
            nc.sync.dma_start(iota_sb[:], io_d[:])
            ident_sb = cpool.tile([P, P], F32)
            nc.sync.dma_start(ident_sb[:], id_d[:])
            xT_sb = cpool.tile([P, NPC], BF16)
            nc.sync.dma_start(xT_sb[:], xT_d[:])
            hTk0 = cpool.tile([P, NPC], BF16)
            hTk1 = cpool.tile([P, NPC], BF16)
            hTks = [hTk0, hTk1]

            aggT = apool.tile([P, NPC], F32)
            aggE = apool.tile([DE1, NPC], F32)

            # persistent gather buffers (explicit rotation); zeroed once so
            # runtime-skipped (padded) rows always hold finite data
            NGB = 4
            gbA_bufs, gbB_bufs = [], []
            for i in range(NGB):
                ga = gpool.tile([P, BA * P], BF16, name=f"gbA{i}")
                nc.vector.memset(ga[:], 0.0)
                gbA_bufs.append(ga)
                gb = gpool.tile([P, BB * P], BF16, name=f"gbB{i}")
                nc.vector.memset(gb[:], 0.0)
                gbB_bufs.append(gb)

            h_own = [drpool.tile([NPC, H], BF16, name=f"h_own{i}")
                     for i in range(2)]
            h_cks = [[drpool.tile([(b - a) * P * NCORES, H], BF16,
                                  addr_space="Shared", name=f"h_ck{i}_{k}")
                      for k, (a, b) in enumerate(chunk_tiles)]
                     for i in range(2)]
            h_fulls = [drpool.tile([NPAD, H], BF16, name=f"h_full{i}")
                       for i in range(2)]

            creg = nc.gpsimd.alloc_register("gcnt")

            for l in range(3):
                src_ap = x_d[:] if l == 0 else h_fulls[(l - 1) % 2][:]
                prevT = xT_sb if l == 0 else hTks[(l - 1) % 2]
                hTk = hTks[l % 2]

                wef_hi = wpool.tile([P, 2 * H], F32, tag="wef_hi")
                nc.sync.dma_start(wef_hi[:], wef_d[l, 0:P, :])
                wef_lo = wpool.tile([DE1, 2 * H], F32, tag="wef_lo")
                nc.sync.dma_start(wef_lo[:], wef_d[l, P:P + DE1, :])
                w2a = wpool.tile([P, H], F32, tag="w2a")
                nc.sync.dma_start(w2a[:], w2_d[l, 0:P, :])
                w2b = wpool.tile([P, H], F32, tag="w2b")
                nc.sync.dma_start(w2b[:], w2_d[l, P:2 * P, :])
                bsb = wpool.tile([P, 3], F32, tag="bsb")
                nc.sync.dma_start(bsb[:], bf_d[l, :, :])

                def dense_window(w, l=l, wef_hi=wef_hi, wef_lo=wef_lo,
                                 w2a=w2a, w2b=w2b, bsb=bsb, hTk=hTk):
                    c0 = w * 512
                    cw = min(512, NPC - c0)
                    ys = []
                    for hf in range(2):
                        psz = psd.tile([P, 512], F32, tag="psz")
                        nc.tensor.matmul(
                            out=psz[:, :cw],
                            lhsT=wef_hi[:, hf * P:(hf + 1) * P],
                            rhs=aggT[:, c0:c0 + cw],
                            start=True, stop=False)
                        nc.tensor.matmul(
                            out=psz[:, :cw],
                            lhsT=wef_lo[:, hf * P:(hf + 1) * P],
                            rhs=aggE[:, c0:c0 + cw],
                            start=False, stop=True)
                        y = dpool.tile([P, 512], F32, tag=f"y{hf}")
                        nc.scalar.activation(
                            out=y[:, :cw], in_=psz[:, :cw], func=Relu,
                            bias=bsb[:, hf:hf + 1], scale=1.0)
                        ys.append(y)
                    psh = psd.tile([P, 512], F32, tag="psh", bufs=1)
                    nc.tensor.matmul(out=psh[:, :cw], lhsT=w2a[:],
                                     rhs=ys[0][:, :cw], start=True, stop=False)
                    nc.tensor.matmul(out=psh[:, :cw], lhsT=w2b[:],
                                     rhs=ys[1][:, :cw], start=False, stop=True)
                    hT = dpool.tile([P, 512], F32, tag="hT")
                    nc.scalar.activation(
                        out=hT[:, :cw], in_=psh[:, :cw],
                        func=(Relu if l < 2 else Identity),
                        bias=bsb[:, 2:3], scale=1.0)
                    if l == 2:
                        nc.sync.dma_start(out_d[:, c0:c0 + cw], hT[:, :cw])
                    else:
                        nc.vector.tensor_copy(hTk[:, c0:c0 + cw], hT[:, :cw])
                        for s in range(cw // P):
                            pst = psd.tile([P, P], F32, tag="pst", bufs=1)
                            nc.tensor.transpose(
                                out=pst[:], in_=hT[:, s * P:(s + 1) * P],
                                identity=ident_sb[:])
                            hr = dpool.tile([P, P], BF16, tag="hr")
                            nc.scalar.copy(out=hr[:], in_=pst[:])
                            nc.sync.dma_start(
                                h_own[l][c0 + s * P:c0 + (s + 1) * P, :], hr[:])
                        if w in chunk_after_w:
                            k = chunk_after_w[w]
                            a, b = chunk_tiles[k]
                            nc.gpsimd.collective_compute(
                                "AllGather",
                                mybir.AluOpType.bypass,
                                replica_groups=[list(range(NCORES))],
                                ins=[h_own[l][a * P:b * P, :].opt()],
                                outs=[h_cks[l][k].opt()],
                            )
                            nc.sync.dma_start(
                                h_fulls[l % 2][:].rearrange(
                                    "(c n) h -> c n h", c=NCORES)[:, a * P:b * P, :],
                                h_cks[l][k][:].rearrange(
                                    "(c n) h -> c n h", c=NCORES),
                            )

                # ---- phase 1 with interleaved dense windows ----
                next_w = 0
                for t in range(TPC):
                    nc.gpsimd.reg_load(creg, cnt_sb[0:1, 2 * t:2 * t + 1])
                    gbA = gbA_bufs[t % NGB]
                    nc.gpsimd.dma_gather(
                        out_ap=gbA[:].rearrange("p (n k) -> p n k", k=P),
                        in_ap=src_ap[0:SA, :],
                        idxs_ap=idxA_sb[:, t * BA * 8:(t + 1) * BA * 8],
                        num_idxs=BA * P,
                        num_idxs_reg=creg,
                        elem_size=H,
                        single_packet=False,
                        queue_num=(2 * t) % 4,
                    )
                    nc.gpsimd.reg_load(creg, cnt_sb[0:1, 2 * t + 1:2 * t + 2])
                    gbB = gbB_bufs[t % NGB]
                    nc.gpsimd.dma_gather(
                        out_ap=gbB[:].rearrange("p (n k) -> p n k", k=P),
                        in_ap=src_ap[SA:NPAD, :],
                        idxs_ap=idxB_sb[:, t * BB * 8:(t + 1) * BB * 8],
                        num_idxs=BB * P,
                        num_idxs_reg=creg,
                        elem_size=H,
                        single_packet=False,
                        queue_num=(2 * t + 1) % 4,
                    )
                    ob = opool.tile([P, BT * P], BF16, tag="ob")
                    nc.vector.tensor_tensor(
                        out=ob[:].rearrange("p (b k) -> p b k", k=P),
                        in0=iota_sb[:].rearrange("p (b k) -> p b k", k=P),
                        in1=dst_loc_sb[:, t * BT:(t + 1) * BT]
                            .to_broadcast([P, BT, P]),
                        op=mybir.AluOpType.is_equal,
                    )
                    ps = psa.tile([P, P], F32, tag="ps", bufs=3)
                    for j in range(BA):
                        nc.tensor.matmul(
                            out=ps[:],
                            lhsT=gbA[:, j * P:(j + 1) * P],
                            rhs=ob[:, j * P:(j + 1) * P],
                            start=(j == 0), stop=False)
                    for j in range(BB):
                        nc.tensor.matmul(
                            out=ps[:],
                            lhsT=gbB[:, j * P:(j + 1) * P],
                            rhs=ob[:, (BA + j) * P:(BA + j + 1) * P],
                            start=False, stop=(j == BB - 1))
                    # agg = psum + h_prev (peeled self-loop)
                    nc.vector.tensor_tensor(
                        out=aggT[:, t * P:(t + 1) * P],
                        in0=ps[:],
                        in1=prevT[:, t * P:(t + 1) * P],
                        op=mybir.AluOpType.add,
                    )
                    if l == 0:
                        eb = eapool.tile([P, BT * DE1], BF16, tag="eb")
                        nc.sync.dma_start(
                            eb[:], ea_d[:, t * BT * DE1:(t + 1) * BT * DE1])
                        pse = psa.tile([DE1, P], F32, tag="pse", bufs=1)
                        for j in range(BT):
                            nc.tensor.matmul(
                                out=pse[:],
                                lhsT=eb[:, j * DE1:(j + 1) * DE1],
                                rhs=ob[:, j * P:(j + 1) * P],
                                start=(j == 0), stop=(j == BT - 1))
                        nc.scalar.copy(out=aggE[:, t * P:(t + 1) * P], in_=pse[:])
                    done_nodes = (t + 1) * P
                    while (next_w + 1) * 512 <= done_nodes or (
                            t == TPC - 1 and next_w < NW):
                        dense_window(next_w)
                        next_w += 1

    nc.compile()
    return nc


# ------------------------------------------------------------------- driver

_LAST_EXEC_NS = None


def kernel(**inputs) -> np.ndarray:
    global _LAST_EXEC_NS
    prep = _prepare(inputs)
    nc = _build_program(prep["TPC"], prep["BA"], prep["BB"], prep["NPAD"])
    res = run_bass_kernel_spmd(nc, prep["in_maps"], list(range(NCORES)))
    _LAST_EXEC_NS = res.exec_time_ns
    out = np.concatenate(
        [np.asarray(res.results[c]["outT"]).T for c in range(NCORES)], 0)
    return out[:prep["N"]].astype(np.float32)
